# revision 1
# baseline (speedup 1.0000x reference)
"""DualPathAttention Trainium2 Bass kernel.

Sharding: batch*head parallel across 8 cores. Core c handles batch b=c//4 and
global heads [4*(c%4), 4*(c%4)+4). Each core computes its 4 heads' dual-path
attention and the partial final projection (its 256 rows of out_w); the host
sums the 4 partials per batch and adds out_b.

Device algorithm per core (layouts chosen so softmax runs along the free axis
and every contraction lands on the partition axis):
  - x^T (D x T) resident in SBUF; projections computed as W^T @ x^T (PE).
  - q^T (256 x T); k^T zero-padded per head to a 128-deep contraction
    (f32r matmuls with K=64 are broken on this HW; K=128 verified good).
  - v,geo_v natural (T x 4*65 with a ones column per head: the U = [v|1]^T@P^T
    accumulation yields the softmax denominator in row 64 for free).
  - Pluecker lines: host gathers/permutes weight columns so the device only
    does 4 projections + elementwise ops; the J6 contraction is folded into
    the write-line weight gather (reversed pair order + signs). The x_prev
    shift is fused into the elementwise multiply APs.
  - Per (head, 512-wide q-block): logits tiles L^T (128k x 512q) via PE, exp
    on ACT (two k-tiles per instruction), causal masking via gpsimd
    affine_select (fill 0 post-exp), U accumulation in PSUM.
  - Combine: denominator row DMA'd from partition 64 to 0 (the custom-DVE
    reciprocal misbehaves at partition 64 on HW), alpha=(1-g)/Dstd,
    beta=g/Dgeo rows broadcast via K=1 matmul, combined^T = Ustd*aB + Ugeo*bB.
  - Final: partial = combined^T.T @ out_w_slice (PE, K=128 zero-padded).
Matmuls run as float32r (fp32 storage, 1 cyc/row at N>=256, ~2.3e-4 rel err).
"""

import os
import numpy as np
import ml_dtypes

import concourse.bass as bass
from concourse import bacc
import concourse.mybir as mybir
import concourse.tile as tile
from concourse.bass_utils import run_bass_kernel_spmd

D, H, B, T = 1024, 16, 2, 2048
DH = 64          # head dim
NH = 4           # heads per core
NCORES = 8
QB = 512         # q block width
KT = 128         # k tile height
NQB = T // QB    # 4
F32 = mybir.dt.float32
F32R = mybir.dt.float32r

PAIRS4 = [(0, 1), (0, 2), (0, 3), (1, 2), (1, 3), (2, 3)]
SIGMA = [1.0, -1.0, 1.0, 1.0, -1.0, 1.0]

TRACE = False            # set by test harness for profiling runs
DEBUG = False            # adds intermediate-dump outputs
LAST_RESULT = None       # BassKernelResults of last run (for exec_time_ns)


def _r(ap):
    return ap.bitcast(F32R)


def _f(ap):
    return ap.bitcast(F32)


def _build_nc():
    nc = bacc.Bacc("TRN2", target_bir_lowering=False, debug=False)

    # ---- DRAM I/O ----
    d_xT = nc.dram_tensor("xT", [D, T], F32R, kind="ExternalInput")
    d_wq = nc.dram_tensor("wq", [D, 256], F32R, kind="ExternalInput")
    d_wk = nc.dram_tensor("wk", [D, 256], F32R, kind="ExternalInput")
    d_wv = nc.dram_tensor("wv", [D, 256], F32R, kind="ExternalInput")
    d_wgv = nc.dram_tensor("wgv", [D, 256], F32R, kind="ExternalInput")
    d_wla = nc.dram_tensor("wla", [D, 64], F32R, kind="ExternalInput")
    d_wlb = nc.dram_tensor("wlb", [D, 64], F32R, kind="ExternalInput")
    d_wlc = nc.dram_tensor("wlc", [D, 64], F32R, kind="ExternalInput")
    d_wld = nc.dram_tensor("wld", [D, 64], F32R, kind="ExternalInput")
    d_wgate = nc.dram_tensor("wgate", [D, 16], F32R, kind="ExternalInput")
    d_outw = nc.dram_tensor("outw", [256, D], F32R, kind="ExternalInput")
    d_bq = nc.dram_tensor("bq", [256, 1], F32, kind="ExternalInput")
    d_bk = nc.dram_tensor("bk", [256, 1], F32, kind="ExternalInput")
    d_bv = nc.dram_tensor("bv", [1, 256], F32R, kind="ExternalInput")
    d_bgv = nc.dram_tensor("bgv", [1, 256], F32R, kind="ExternalInput")
    d_bgate = nc.dram_tensor("bgate", [16, 1], F32, kind="ExternalInput")
    d_sbc = nc.dram_tensor("sbc", [64, 1], F32, kind="ExternalInput")
    d_ssel = nc.dram_tensor("ssel", [128, 64], F32R, kind="ExternalInput")
    d_ones = nc.dram_tensor("ones", [65, 128], F32R, kind="ExternalInput")
    d_gsel = nc.dram_tensor("gsel", [16, 1], F32R, kind="ExternalInput")
    d_vones = nc.dram_tensor("vones", [128, 64], F32R, kind="ExternalInput")
    d_zrow = nc.dram_tensor("zrow", [1, T], F32R, kind="ExternalInput")
    d_partial = nc.dram_tensor("partial", [T, D], F32, kind="ExternalOutput")
    if DEBUG:
        d_dbg_us = nc.dram_tensor("dbg_us", [65, 4, QB], F32, kind="ExternalOutput")
        d_dbg_ug = nc.dram_tensor("dbg_ug", [65, 4, QB], F32, kind="ExternalOutput")
        d_dbg_ab = nc.dram_tensor("dbg_ab", [64, 4, QB], F32, kind="ExternalOutput")
        d_dbg_jw = nc.dram_tensor("dbg_jw", [128, T], F32, kind="ExternalOutput")
        d_dbg_rl = nc.dram_tensor("dbg_rl", [128, T], F32, kind="ExternalOutput")
        d_dbg_comb = nc.dram_tensor("dbg_comb", [128, 4, T], F32, kind="ExternalOutput")

    AF = mybir.ActivationFunctionType
    OP = mybir.AluOpType

    def zfill(dst_ap, rows, cols):
        """Zero-fill an SBUF region via broadcast DMA from the zeros row."""
        src = bass.AP(tensor=d_zrow, offset=0, ap=[[0, rows], [1, cols]])
        nc.sync.dma_start(dst_ap, src)

    with tile.TileContext(nc, linearize=bool(int(os.environ.get('KLIN', '0')))) as tc:
        with (
            tc.tile_pool(name="const", bufs=1) as cpool,
            tc.tile_pool(name="pers1", bufs=1) as pers1,
            tc.tile_pool(name="psS", bufs=4, space=bass.MemorySpace.PSUM) as psS,
            tc.tile_pool(name="psL", bufs=2, space=bass.MemorySpace.PSUM) as psL,
        ):
            # ---- constants ----
            ones_all = cpool.tile([65, 128], F32R)   # ones (K=1 bcast/bias lhsT)
            nc.sync.dma_start(ones_all[:], d_ones[:])
            gsel = cpool.tile([16, 1], F32R)         # gate mean selector (1/16)
            nc.sync.dma_start(gsel[:], d_gsel[:])
            ssel = cpool.tile([128, 64], F32R)       # sumsq group selector (padded)
            nc.sync.dma_start(ssel[:], d_ssel[:])

            jwT = pers1.tile([128, T], F32R)   # head h at partitions [32h, 32h+6)
            rlT = pers1.tile([128, T], F32R)

            # right-side stack pool is opened after the lines phase (it
            # persists past xp's close; left/right are independent LIFO stacks)
            pers2_cm = tc.tile_pool(name="pers2", bufs=1, side="right")
            pers2 = None
            try:
                with tc.tile_pool(name="xp", bufs=1) as xp:
                    xT_sb = xp.tile([128, 8, T], F32R)
                    for ko in range(8):
                        nc.sync.dma_start(
                            xT_sb[:, ko, :], d_xT[128 * ko:128 * (ko + 1), :])

                    # ---------- A1: Pluecker lines ----------
                    # operand layout (64 x T): rows 0:24 write-path (+pad8),
                    # rows 32:56 read-path (+pad8). A/C get the x_prev shift.
                    with (
                        tc.tile_pool(name="wl", bufs=1) as wl,
                        tc.tile_pool(name="lines", bufs=1) as lnp,
                    ):
                        sbc_sb = wl.tile([64, 1], F32)
                        nc.sync.dma_start(sbc_sb[:], d_sbc[:])

                        def _project(dst, wsb):
                            for tb in range(NQB):
                                ps = psS.tile([64, QB], F32, tag="s", name="lps")
                                for kc in range(8):
                                    nc.tensor.matmul(
                                        ps[:], _r(wsb[:, kc, :]),
                                        _r(xT_sb[:, kc, QB * tb:QB * (tb + 1)]),
                                        start=(kc == 0), stop=(kc == 7))
                                nc.scalar.copy(dst[:, QB * tb:QB * (tb + 1)], ps[:])

                        def _product(t, d_w1, d_w2, wtag):
                            w1sb = wl.tile([128, 8, 64], F32R, tag="w1",
                                           name=wtag + "1")
                            w2sb = wl.tile([128, 8, 64], F32R, tag="w2",
                                           name=wtag + "2")
                            nc.sync.dma_start(
                                w1sb[:], d_w1[:].rearrange("(k p) c -> p k c", p=128))
                            nc.sync.dma_start(
                                w2sb[:], d_w2[:].rearrange("(k p) c -> p k c", p=128))
                            PX = lnp.tile([64, T], F32, tag="a", name="PX")
                            PY = lnp.tile([64, T], F32, tag="b", name="PY")
                            _project(PX, w1sb)
                            _project(PY, w2sb)
                            nc.vector.tensor_mul(
                                _r(t[0:32, 1:T]), PX[0:32, 0:T - 1], PY[0:32, 1:T])
                            # zero col 0 via always-false affine_select
                            nc.gpsimd.affine_select(
                                out=_r(t[0:32, 0:1]), in_=_r(t[0:32, 1:2]),
                                compare_op=OP.is_gt, fill=0.0,
                                base=0, pattern=[[0, 1]], channel_multiplier=0)
                            nc.vector.tensor_mul(
                                _r(t[32:64, :]), PX[32:64, :], PY[32:64, :])

                        t1 = lnp.tile([64, T], F32, tag="e")
                        t2 = lnp.tile([128, T], F32, tag="f")  # padded ssel rhs
                        zfill(_r(t2[64:128, :]), 64, T)
                        _product(t1, d_wla, d_wlb, "wab")
                        _product(t2, d_wlc, d_wld, "wcd")
                        nc.vector.tensor_sub(t1[:], t1[:], t2[0:64, :])  # lines_u
                        nc.scalar.square(_r(t2[0:64, :]), t1[:])         # squares

                        ssq = lnp.tile([64, T], F32, tag="a")
                        for tb in range(NQB):
                            ps = psS.tile([64, QB], F32, tag="s")
                            nc.tensor.matmul(
                                ps[:], _r(ssel[:]),
                                _r(t2[:, QB * tb:QB * (tb + 1)]),
                                start=True, stop=True)
                            nc.vector.tensor_scalar_max(
                                out=ssq[:, QB * tb:QB * (tb + 1)], in0=ps[:],
                                scalar1=1e-24)
                        rt = lnp.tile([64, T], F32, tag="b")
                        nc.scalar.sqrt(rt[:], ssq[:])
                        inv = lnp.tile([64, T], F32, tag="a")
                        nc.vector.reciprocal_approx_fast(out=inv[:], in_=rt[:])
                        # fold inc_scale into read-line norms (rows 0:32 are 1.0)
                        nc.vector.tensor_scalar_mul(
                            out=inv[:], in0=inv[:], scalar1=sbc_sb[:, 0:1])
                        nc.vector.tensor_mul(_r(t1[:]), t1[:], inv[:])   # lines_n

                        # scatter to 32-aligned per-head layout via DMA
                        for h in range(NH):
                            nc.sync.dma_start(
                                out=jwT[32 * h:32 * h + 6, :],
                                in_=_r(t1[6 * h:6 * h + 6, :]))
                            nc.sync.dma_start(
                                out=rlT[32 * h:32 * h + 6, :],
                                in_=_r(t1[32 + 6 * h:32 + 6 * h + 6, :]))

                    pers2 = pers2_cm.__enter__()
                    qT = pers2.tile([128, 2, T], F32R)
                    # k^T padded: head h at partitions [64*(h%2), +64), zeros else
                    kTp = pers2.tile([128, NH, T], F32R)
                    vplus = pers2.tile([128, 16, NH * 65], F32R)
                    gvplus = pers2.tile([128, 16, NH * 65], F32R)
                    g_row = pers2.tile([1, T], F32)
                    g1m_row = pers2.tile([1, T], F32)
                    for h in range(NH):
                        zfill(kTp[64 * ((h + 1) % 2):64 * ((h + 1) % 2) + 64, h, :],
                              64, T)
                    # ones columns of v/gv (col 64 of each head's 65-wide group)
                    nc.sync.dma_start(
                        vplus[:].rearrange(
                            "p t (h c) -> p t h c", c=65)[:, :, :, 64:65],
                        d_vones[:].rearrange("p (t h) -> p t h", h=NH))
                    nc.sync.dma_start(
                        gvplus[:].rearrange(
                            "p t (h c) -> p t h c", c=65)[:, :, :, 64:65],
                        d_vones[:].rearrange("p (t h) -> p t h", h=NH))

                    # ---------- A2a: q^T / k^T ----------
                    with tc.tile_pool(name="w2a", bufs=1) as w2a:
                        wq_sb = w2a.tile([128, 8, 256], F32R)
                        wk_sb = w2a.tile([128, 8, 256], F32R)
                        nc.sync.dma_start(
                            wq_sb[:], d_wq[:].rearrange("(k p) c -> p k c", p=128))
                        nc.sync.dma_start(
                            wk_sb[:], d_wk[:].rearrange("(k p) c -> p k c", p=128))
                        bq_sb = w2a.tile([128, 2], F32)
                        bk_sb = w2a.tile([128, 2], F32)
                        nc.sync.dma_start(
                            bq_sb[:], d_bq[:].rearrange("(m p) o -> p (m o)", p=128))
                        nc.sync.dma_start(
                            bk_sb[:], d_bk[:].rearrange("(m p) o -> p (m o)", p=128))

                        for mc in range(2):
                            for (wsb, bias, isq) in ((wq_sb, bq_sb, True),
                                                     (wk_sb, bk_sb, False)):
                                pss = [psS.tile([128, QB], F32, tag="s",
                                                name=f"pss{tb}")
                                       for tb in range(NQB)]
                                for kc in range(8):
                                    for tb in range(NQB):
                                        nc.tensor.matmul(
                                            pss[tb][:],
                                            _r(wsb[:, kc, 128 * mc:128 * (mc + 1)]),
                                            _r(xT_sb[:, kc, QB * tb:QB * (tb + 1)]),
                                            start=(kc == 0), stop=(kc == 7))
                                for tb in range(NQB):
                                    sl = slice(QB * tb, QB * (tb + 1))
                                    if isq:
                                        nc.vector.tensor_scalar_add(
                                            out=qT[:, mc, sl], in0=pss[tb][:],
                                            scalar1=bias[:, mc:mc + 1])
                                    else:
                                        # k^T: head 2mc at rows 0:64 of its slot,
                                        # head 2mc+1 at rows 64:128 of its slot
                                        nc.vector.tensor_scalar_add(
                                            out=kTp[0:64, 2 * mc, sl],
                                            in0=pss[tb][0:64, :],
                                            scalar1=bias[0:64, mc:mc + 1])
                                        nc.vector.tensor_scalar_add(
                                            out=kTp[64:128, 2 * mc + 1, sl],
                                            in0=pss[tb][64:128, :],
                                            scalar1=bias[64:128, mc:mc + 1])

                    # ---------- A2b: v / geo_v / gate ----------
                    with tc.tile_pool(name="w2b", bufs=1) as w2b:
                        wv_sb = w2b.tile([128, 8, 256], F32R)
                        wgv_sb = w2b.tile([128, 8, 256], F32R)
                        nc.sync.dma_start(
                            wv_sb[:], d_wv[:].rearrange("(k p) c -> p k c", p=128))
                        nc.sync.dma_start(
                            wgv_sb[:], d_wgv[:].rearrange("(k p) c -> p k c", p=128))
                        wgate_sb = w2b.tile([128, 8, 16], F32R)
                        nc.sync.dma_start(
                            wgate_sb[:],
                            d_wgate[:].rearrange("(k p) c -> p k c", p=128))
                        bv_sb = w2b.tile([1, 256], F32R)
                        bgv_sb = w2b.tile([1, 256], F32R)
                        bgate_sb = w2b.tile([16, 1], F32)
                        nc.sync.dma_start(bv_sb[:], d_bv[:])
                        nc.sync.dma_start(bgv_sb[:], d_bgv[:])
                        nc.sync.dma_start(bgate_sb[:], d_bgate[:])

                        for (dst, wsb, bias) in ((vplus, wv_sb, bv_sb),
                                                 (gvplus, wgv_sb, bgv_sb)):
                            for ti in range(16):
                                ps = psS.tile([128, 256], F32, tag="s")
                                nc.tensor.matmul(ps[:], _r(ones_all[0:1, :]),
                                                 bias[:],
                                                 start=True, stop=False)
                                for kc in range(8):
                                    nc.tensor.matmul(
                                        ps[:],
                                        _r(xT_sb[:, kc, 128 * ti:128 * (ti + 1)]),
                                        _r(wsb[:, kc, :]),
                                        start=False, stop=(kc == 7))
                                nc.vector.tensor_copy(
                                    dst[:, ti, :].rearrange(
                                        "p (h c) -> p h c", c=65)[:, :, 0:64],
                                    ps[:].rearrange("p (h c) -> p h c", c=64))

                        # gate: mean_h sigmoid(gate_w^T x + b), all at partition 0
                        for tb in range(NQB):
                            ps = psS.tile([16, QB], F32, tag="s")
                            for kc in range(8):
                                nc.tensor.matmul(
                                    ps[:], _r(wgate_sb[:, kc, :]),
                                    _r(xT_sb[:, kc, QB * tb:QB * (tb + 1)]),
                                    start=(kc == 0), stop=(kc == 7))
                            gsig = w2b.tile([16, QB], F32R, tag="gsig")
                            nc.scalar.activation(
                                out=gsig[:], in_=ps[:],
                                func=AF.Sigmoid, bias=bgate_sb[:, 0:1], scale=1.0)
                            psg = psS.tile([1, QB], F32, tag="s")
                            nc.tensor.matmul(psg[:], _r(gsel[:]), gsig[:],
                                             start=True, stop=True)
                            nc.vector.tensor_copy(
                                g_row[:, QB * tb:QB * (tb + 1)], psg[:])
                        nc.vector.tensor_scalar(
                            out=g1m_row[:], in0=g_row[:],
                            scalar1=-1.0, scalar2=1.0, op0=OP.mult, op1=OP.add)

                # ---------- Phase B: dual-path attention ----------
                with tc.tile_pool(name="combp", bufs=1) as combp:
                    # combined^T padded to K=128 for the final projection
                    comb = combp.tile([128, NH, T], F32R)
                    for hc in range(NH):
                        zfill(comb[64:128, hc, :], 64, T)

                    with (
                        tc.tile_pool(name="ptiles", bufs=5) as pp,
                        tc.tile_pool(name="abt", bufs=6) as abp,
                        tc.tile_pool(name="rows", bufs=4) as rowp,
                    ):
                        for p in range(NH):
                            for j in range(NQB):
                                qsl = slice(QB * j, QB * (j + 1))
                                nkt = 4 * (j + 1)
                                Us = psS.tile([65, QB], F32, tag="s")
                                Ug = psS.tile([65, QB], F32, tag="s")
                                for half in range(nkt // 2):
                                    Ls = psL.tile([128, 2, QB], F32, tag="L")
                                    Lg = psL.tile([128, 2, QB], F32, tag="L")
                                    for i in range(2):
                                        kt = 2 * half + i
                                        ksl = slice(KT * kt, KT * (kt + 1))
                                        nc.tensor.matmul(
                                            Ls[:, i, :],
                                            _r(kTp[:, p, ksl]),
                                            _r(qT[:, p // 2, qsl]),
                                            start=True, stop=True)
                                        nc.tensor.matmul(
                                            Lg[:, i, :],
                                            _r(jwT[32 * p:32 * p + 6, ksl]),
                                            _r(rlT[32 * p:32 * p + 6, qsl]),
                                            start=True, stop=True,
                                            tile_position=(32 * p, 0))
                                    Ps = pp.tile([128, 2, QB], F32R, tag="P")
                                    Pg = pp.tile([128, 2, QB], F32R, tag="P")
                                    nc.scalar.activation(Ps[:], Ls[:], AF.Exp)
                                    nc.scalar.activation(Pg[:], Lg[:], AF.Exp)
                                    for i in range(2):
                                        kt = 2 * half + i
                                        m = kt - 4 * j
                                        if m >= 0:
                                            w = KT * (m + 1)
                                            for Px in (Ps, Pg):
                                                # keep where q_r - k_r - 128m >= 0
                                                nc.gpsimd.affine_select(
                                                    out=Px[:, i, 0:w],
                                                    in_=Px[:, i, 0:w],
                                                    compare_op=OP.is_ge, fill=0.0,
                                                    base=-KT * m, pattern=[[1, w]],
                                                    channel_multiplier=-1)
                                        nc.tensor.matmul(
                                            Us[:],
                                            _r(vplus[:, kt, 65 * p:65 * p + 65]),
                                            _r(Ps[:, i, :]),
                                            start=(kt == 0), stop=(kt == nkt - 1))
                                        nc.tensor.matmul(
                                            Ug[:],
                                            _r(gvplus[:, kt, 65 * p:65 * p + 65]),
                                            _r(Pg[:, i, :]),
                                            start=(kt == 0), stop=(kt == nkt - 1))
                                if DEBUG and p == 0:
                                    us_sb = abp.tile([65, QB], F32, tag="dbg",
                                                     name="us_sb")
                                    ug_sb = abp.tile([65, QB], F32, tag="dbg",
                                                     name="ug_sb")
                                    nc.vector.tensor_copy(us_sb[:], Us[:])
                                    nc.vector.tensor_copy(ug_sb[:], Ug[:])
                                    nc.sync.dma_start(d_dbg_us[:, j, :], us_sb[:])
                                    nc.sync.dma_start(d_dbg_ug[:, j, :], ug_sb[:])
                                # combine: alpha=(1-g)/Dstd, beta=g/Dgeo.
                                # The denominator row sits at psum partition 64;
                                # stage to SBUF then DMA to partition 0 (the
                                # custom-DVE recip misbehaves at partition 64).
                                dst64 = rowp.tile([65, QB], F32, tag="rd")
                                dgt64 = rowp.tile([65, QB], F32, tag="rd")
                                nc.vector.tensor_copy(dst64[64:65, :], Us[64:65, :])
                                nc.vector.tensor_copy(dgt64[64:65, :], Ug[64:65, :])
                                d0 = rowp.tile([1, QB], F32, tag="r0")
                                e0 = rowp.tile([1, QB], F32, tag="r0")
                                nc.sync.dma_start(d0[:], dst64[64:65, :])
                                nc.sync.dma_start(e0[:], dgt64[64:65, :])
                                rs = rowp.tile([1, QB], F32, tag="r0")
                                rg = rowp.tile([1, QB], F32, tag="r0")
                                nc.vector.reciprocal_approx_fast(
                                    out=rs[:], in_=d0[:])
                                nc.vector.reciprocal_approx_fast(
                                    out=rg[:], in_=e0[:])
                                ar = rowp.tile([1, QB], F32R, tag="r1")
                                br = rowp.tile([1, QB], F32R, tag="r1")
                                nc.vector.tensor_mul(
                                    ar[:], rs[:], g1m_row[:, qsl])
                                nc.vector.tensor_mul(
                                    br[:], rg[:], g_row[:, qsl])
                                psA = psS.tile([64, QB], F32, tag="s")
                                psB = psS.tile([64, QB], F32, tag="s")
                                nc.tensor.matmul(
                                    psA[:], _r(ones_all[0:1, 0:64]), ar[:],
                                    start=True, stop=True)
                                nc.tensor.matmul(
                                    psB[:], _r(ones_all[0:1, 0:64]), br[:],
                                    start=True, stop=True)
                                aB = abp.tile([64, QB], F32, tag="ab")
                                bB = abp.tile([64, QB], F32, tag="ab")
                                nc.vector.tensor_copy(aB[:], psA[:])
                                if DEBUG and p == 0:
                                    nc.sync.dma_start(d_dbg_ab[:, j, :], aB[:])
                                nc.vector.tensor_copy(bB[:], psB[:])
                                u1 = abp.tile([64, QB], F32, tag="ab")
                                u2 = abp.tile([64, QB], F32, tag="ab")
                                nc.vector.tensor_mul(u1[:], Us[0:64, :], aB[:])
                                nc.vector.tensor_mul(u2[:], Ug[0:64, :], bB[:])
                                nc.vector.tensor_add(
                                    comb[0:64, p, qsl], u1[:], u2[:])

                    if DEBUG:
                        nc.sync.dma_start(d_dbg_jw[:], _f(jwT[:]))
                        nc.sync.dma_start(d_dbg_rl[:], _f(rlT[:]))
                        nc.sync.dma_start(d_dbg_comb[:], _f(comb[:]))

                    # ---------- Phase C: final projection ----------
                    with tc.tile_pool(name="outs", bufs=4) as op_:
                        outw_sb = op_.tile([128, NH, D], F32R, tag="w", bufs=1)
                        nc.sync.dma_start(
                            outw_sb[0:64, :, :],
                            d_outw[:].rearrange("(h p) e -> p h e", p=64))
                        for hc in range(NH):
                            zfill(outw_sb[64:128, hc, :], 64, D)
                        for qt in range(16):
                            for et in range(2):
                                ps = psS.tile([128, QB], F32, tag="s")
                                for hc in range(NH):
                                    nc.tensor.matmul(
                                        ps[:],
                                        _r(comb[:, hc, 128 * qt:128 * (qt + 1)]),
                                        _r(outw_sb[:, hc, QB * et:QB * (et + 1)]),
                                        start=(hc == 0), stop=(hc == 3))
                                ot = op_.tile([128, QB], F32, tag="o")
                                nc.scalar.copy(ot[:], ps[:])
                                nc.sync.dma_start(
                                    d_partial[128 * qt:128 * (qt + 1),
                                              QB * et:QB * (et + 1)],
                                    ot[:])
            finally:
                if pers2 is not None:
                    pers2_cm.__exit__(None, None, None)
    nc.compile()
    return nc


_nc_cache = None


def _get_nc():
    global _nc_cache
    if _nc_cache is None:
        _nc_cache = _build_nc()
    return _nc_cache


def _prep_core_inputs(inputs, core):
    b = core // 4
    h0 = (core % 4) * 4
    f = np.float32
    qkv_w, qkv_b = inputs['qkv_w'], inputs['qkv_b']
    scale = DH ** -0.5
    s = slice(h0 * DH, h0 * DH + NH * DH)
    ac = np.ascontiguousarray

    # Operand layout: 64 cols = [write(24)+pad8 | read(24)+pad8]; A/C from w1
    # (shifted = x_prev side), B/D from the w2/read counterparts.
    WLA = np.zeros((D, 64), f); WLB = np.zeros((D, 64), f)
    WLC = np.zeros((D, 64), f); WLD = np.zeros((D, 64), f)
    w1w, w2w = inputs['w1_write'], inputs['w2_write']
    w1r, w2r = inputs['w1_read'], inputs['w2_read']
    for h in range(NH):
        gh = h0 + h
        for jj in range(6):
            i_, j_ = PAIRS4[5 - jj]
            WLA[:, 0 + h * 6 + jj] = w1w[:, gh * 4 + i_] * SIGMA[jj]    # A_w
            WLB[:, 0 + h * 6 + jj] = w2w[:, gh * 4 + j_]                # B_w
            WLC[:, 0 + h * 6 + jj] = w1w[:, gh * 4 + j_] * SIGMA[jj]    # C_w
            WLD[:, 0 + h * 6 + jj] = w2w[:, gh * 4 + i_]                # D_w
        for pp in range(6):
            i_, j_ = PAIRS4[pp]
            WLA[:, 32 + h * 6 + pp] = w1r[:, gh * 4 + i_]               # A_r
            WLB[:, 32 + h * 6 + pp] = w2r[:, gh * 4 + j_]               # B_r
            WLC[:, 32 + h * 6 + pp] = w1r[:, gh * 4 + j_]               # C_r
            WLD[:, 32 + h * 6 + pp] = w2r[:, gh * 4 + i_]               # D_r

    ssel = np.zeros((128, 64), f)
    for half in (0, 32):
        for h in range(NH):
            g = slice(half + 6 * h, half + 6 * h + 6)
            ssel[g, g] = 1.0
    sbc = np.ones((64, 1), f)
    sbc[32:56, 0] = np.repeat(inputs['inc_scale'][h0:h0 + NH], 6).astype(f)

    return {
        'xT': ac(np.asarray(inputs['x'][b], f).T),
        'wq': ac((qkv_w[:, 0 * D:1 * D][:, s] * scale).astype(f)),
        'wk': ac(qkv_w[:, 1 * D:2 * D][:, s].astype(f)),
        'wv': ac(qkv_w[:, 2 * D:3 * D][:, s].astype(f)),
        'wgv': ac(inputs['geo_w'][:, s].astype(f)),
        'wla': WLA, 'wlb': WLB, 'wlc': WLC, 'wld': WLD,
        'wgate': ac(inputs['gate_w'].astype(f)),
        'outw': ac(inputs['out_w'][s, :].astype(f)),
        'bq': ac((qkv_b[0 * D:1 * D][s] * scale).astype(f).reshape(256, 1)),
        'bk': ac(qkv_b[1 * D:2 * D][s].astype(f).reshape(256, 1)),
        'bv': ac(qkv_b[2 * D:3 * D][s].astype(f).reshape(1, 256)),
        'bgv': ac(inputs['geo_b'][s].astype(f).reshape(1, 256)),
        'bgate': ac(inputs['gate_b'].astype(f).reshape(16, 1)),
        'sbc': sbc,
        'ssel': ssel,
        'ones': np.ones((65, 128), f),
        'gsel': np.full((16, 1), 1.0 / 16.0, f),
        'vones': np.ones((128, 64), f),
        'zrow': np.zeros((1, T), f),
    }


def kernel(**inputs):
    global LAST_RESULT
    inputs = {k: np.asarray(v) for k, v in inputs.items()}
    nc = _get_nc()
    in_maps = [_prep_core_inputs(inputs, c) for c in range(NCORES)]
    res = run_bass_kernel_spmd(nc, in_maps, core_ids=list(range(NCORES)),
                               trace=TRACE)
    LAST_RESULT = res
    out = np.zeros((B, T, D), np.float32)
    for c in range(NCORES):
        out[c // 4] += res.results[c]['partial']
    out += np.asarray(inputs['out_b'], np.float32)[None, None, :]
    return out



# revision 7
# speedup vs baseline: 1.2905x; 1.2905x over previous
"""DualPathAttention Trainium2 Bass kernel (v2 — pipelined, bf16).

Sharding: batch*head parallel across 8 cores. Core c handles batch b=c//4 and
global heads [4*(c%4), 4*(c%4)+4). Each core computes its 4 heads' dual-path
attention and the partial final projection (its 256 rows of out_w); the host
sums the 4 partials per batch and adds out_b.

v2 changes vs baseline (577us):
  - All matmuls bf16 (same 1 col/cyc PE rate as f32r at N>=256, but half the
    DMA/SBUF traffic and LDWEIGHTS time). PSUM stays f32; line norm math f32.
  - Single-pass pipeline: lines prologue (overlapped with the x DMA stream via
    kc-outer matmuls over all 4 q-blocks), then per q-block tb: projections
    (PE-heavy) -> attention j=tb (ACT-heavy) -> final projection of block tb.
    A(tb+1) PE work has no deps on B(j=tb), so the PE runs ahead while ACT
    chews the exps -> keeps the PE at the 2.4GHz p-state (idle drops it to
    1.2GHz).
  - x_prev shift via a host-padded leading zero column in xT (2049 cols);
    write-path projections read one column to the left. Projection staging of
    the shifted operands (PXas/PXcs) makes the shift work across blocks.
  - Softmax denominator rides at *row 0* of the AV matmul (ones in column 0 of
    each v group), so the reciprocal runs at partition 0 directly -- no
    partition-shift DMAs in the combine (baseline burned 64 Sync DMAs there).
  - Diagonal tiles narrowed: logits/exp/mask/AV only cover cols >= 128*m of
    the diagonal q-sub-block (saves ~15% PE cols and ACT elems; masks shrink
    to fixed 128-wide affine_selects).
  - No strided elem-4 constant-fill DMAs: zero/one fills via engine memsets,
    out_w host-padded to the [128,4,1024] lhsT layout.
"""

import os
import numpy as np
import ml_dtypes

import concourse.bass as bass
from concourse import bacc
import concourse.mybir as mybir
import concourse.tile as tile
from concourse.bass_utils import run_bass_kernel_spmd

D, H, B, T = 1024, 16, 2, 2048
DH = 64          # head dim
NH = 4           # heads per core
NCORES = 8
QB = 512         # q block width
KT = 128         # k tile height
NQB = T // QB    # 4
NKC = 8          # 128-deep chunks of D
F32 = mybir.dt.float32
BF16 = mybir.dt.bfloat16

PAIRS4 = [(0, 1), (0, 2), (0, 3), (1, 2), (1, 3), (2, 3)]
SIGMA = [1.0, -1.0, 1.0, 1.0, -1.0, 1.0]

TRACE = False            # set by test harness for profiling runs
DEBUG = False
LAST_RESULT = None       # BassKernelResults of last run (for exec_time_ns)


def _build_nc():
    nc = bacc.Bacc("TRN2", target_bir_lowering=False, debug=False)

    # ---- DRAM I/O (host pre-shuffles everything into SBUF layouts) ----
    d_xT = nc.dram_tensor("xT", [128, NKC * (T + 1)], BF16, kind="ExternalInput")
    d_wla = nc.dram_tensor("wla", [128, NKC * 64], BF16, kind="ExternalInput")
    d_wlb = nc.dram_tensor("wlb", [128, NKC * 64], BF16, kind="ExternalInput")
    d_wlc = nc.dram_tensor("wlc", [128, NKC * 64], BF16, kind="ExternalInput")
    d_wld = nc.dram_tensor("wld", [128, NKC * 64], BF16, kind="ExternalInput")
    d_wq = nc.dram_tensor("wq", [128, NKC * 256], BF16, kind="ExternalInput")
    d_wk = nc.dram_tensor("wk", [128, NKC * 256], BF16, kind="ExternalInput")
    d_wvg = nc.dram_tensor("wvg", [128, NKC * 512], BF16, kind="ExternalInput")
    d_wgate = nc.dram_tensor("wgate", [128, NKC * 16], BF16, kind="ExternalInput")
    d_outw = nc.dram_tensor("outw", [128, NH * D], BF16, kind="ExternalInput")
    d_ssel = nc.dram_tensor("ssel", [128, 64], BF16, kind="ExternalInput")
    d_bq = nc.dram_tensor("bq", [128, 2], F32, kind="ExternalInput")
    d_bk = nc.dram_tensor("bk", [128, 2], F32, kind="ExternalInput")
    d_bvg = nc.dram_tensor("bvg", [1, 512], BF16, kind="ExternalInput")
    d_bgate = nc.dram_tensor("bgate", [16, 1], F32, kind="ExternalInput")
    d_sbc = nc.dram_tensor("sbc", [64, 1], F32, kind="ExternalInput")
    d_partial = nc.dram_tensor("partial", [T, D], F32, kind="ExternalOutput")

    AF = mybir.ActivationFunctionType
    OP = mybir.AluOpType

    with tile.TileContext(nc, linearize=bool(int(os.environ.get('KLIN', '0')))) as tc:
        with (
            tc.tile_pool(name="const", bufs=1) as cpool,
            tc.tile_pool(name="pers", bufs=1) as pers,
        ):
            # ---- persistent tiles ----
            xT_sb = cpool.tile([128, NKC, T + 1], BF16)
            wla_sb = cpool.tile([128, NKC, 64], BF16)
            wlb_sb = cpool.tile([128, NKC, 64], BF16)
            wlc_sb = cpool.tile([128, NKC, 64], BF16)
            wld_sb = cpool.tile([128, NKC, 64], BF16)
            wq_sb = cpool.tile([128, NKC, 256], BF16)
            wk_sb = cpool.tile([128, NKC, 256], BF16)
            wvg_sb = cpool.tile([128, NKC, 512], BF16)
            wgate_sb = cpool.tile([128, NKC, 16], BF16)
            outw_sb = cpool.tile([128, NH, D], BF16)
            ssel_sb = cpool.tile([128, 64], BF16)
            bq_sb = cpool.tile([128, 2], F32)
            bk_sb = cpool.tile([128, 2], F32)
            bvg_sb = cpool.tile([1, 512], BF16)
            bgate_sb = cpool.tile([16, 1], F32)
            sbc_sb = cpool.tile([64, 1], F32)
            ones_bf = cpool.tile([1, 128], BF16)
            gsel = cpool.tile([16, 1], BF16)

            qT = pers.tile([128, 2, T], BF16)
            kTp = pers.tile([128, NH, T], BF16)   # head h: rows [64*(h%2),+64)
            vplus = pers.tile([128, 16, NH * 65], BF16)  # group col0 = ones
            gvplus = pers.tile([128, 16, NH * 65], BF16)
            jwT = pers.tile([128, T], BF16)       # head h at rows [32h, 32h+6)
            rlT = pers.tile([128, T], BF16)
            g_row = pers.tile([1, T], F32)
            g1m_row = pers.tile([1, T], F32)
            comb = pers.tile([128, 2, NH, QB], BF16)  # rows 1:65 live

            # ---- DMA issue order: lines weights, x stream, the rest ----
            for d_w, sb in ((d_wla, wla_sb), (d_wlb, wlb_sb),
                            (d_wlc, wlc_sb), (d_wld, wld_sb)):
                nc.sync.dma_start(sb[:], d_w[:])
            for kc in range(NKC):
                nc.sync.dma_start(xT_sb[:, kc, :],
                                  d_xT[:, (T + 1) * kc:(T + 1) * (kc + 1)])
            nc.sync.dma_start(wq_sb[:], d_wq[:])
            nc.sync.dma_start(wk_sb[:], d_wk[:])
            nc.sync.dma_start(wvg_sb[:], d_wvg[:])
            nc.sync.dma_start(wgate_sb[:], d_wgate[:])
            nc.sync.dma_start(outw_sb[:], d_outw[:])
            nc.sync.dma_start(ssel_sb[:], d_ssel[:])
            nc.sync.dma_start(bq_sb[:], d_bq[:])
            nc.sync.dma_start(bk_sb[:], d_bk[:])
            nc.sync.dma_start(bvg_sb[:], d_bvg[:])
            nc.sync.dma_start(bgate_sb[:], d_bgate[:])
            nc.sync.dma_start(sbc_sb[:], d_sbc[:])

            # ---- constant fills (gpsimd; off every critical path at t0) ----
            nc.gpsimd.memset(ones_bf[:], 1.0)
            nc.gpsimd.memset(gsel[:], 1.0 / 16.0)
            for hh in range(NH):
                nc.gpsimd.memset(kTp[64 * ((hh + 1) % 2):64 * ((hh + 1) % 2) + 64,
                                     hh, :], 0.0)
            for vp in (vplus, gvplus):
                nc.gpsimd.memset(
                    vp[:].rearrange("p t (h c) -> p t h c", c=65)[:, :, :, 0:1],
                    1.0)
            # rows 65:128 never written by the combine (row 64 is, later —
            # write-after-memset order is tracked); rows 0 and 65:127 hold
            # finite junk/zero that phase C multiplies by out_w's zero rows.
            nc.gpsimd.memset(comb[64:128, :, :, :], 0.0)

            # ================= Prologue: Pluecker lines =================
            # Operand layout (64 x T): rows 0:24 write-path (+pad8),
            # rows 32:56 read-path (+pad8). A/C projections take the x_prev
            # shift (write rows only), realized by reading the staged
            # projection one column to the left (xT has a leading zero col).
            with (
                tc.tile_pool(name="lines", bufs=1) as lnp,
                tc.tile_pool(name="lnsub", bufs=2) as lns,
                tc.tile_pool(name="psPro", bufs=8,
                             space=bass.MemorySpace.PSUM) as psp,
            ):
                PXas = lnp.tile([64, T + 1], F32)   # staged A-side (wla)
                PXcs = lnp.tile([64, T + 1], F32)   # staged C-side (wlc)
                t1 = lnp.tile([64, T], F32)         # lines_u then reused
                sq = lnp.tile([128, T], BF16)       # squares (K-padded)
                nc.gpsimd.memset(PXas[:, 0:1], 0.0)
                nc.gpsimd.memset(PXcs[:, 0:1], 0.0)
                nc.gpsimd.memset(sq[64:128, :], 0.0)

                def _lines_round(wsb1, wsb2, stage_dst, emit_products):
                    pxs = [psp.tile([64, QB], F32, tag="pro", name=f"px{t}")
                           for t in range(NQB)]
                    pys = [psp.tile([64, QB], F32, tag="pro", name=f"py{t}")
                           for t in range(NQB)]
                    for kc in range(NKC):
                        for tb in range(NQB):
                            sl = slice(1 + QB * tb, 1 + QB * (tb + 1))
                            nc.tensor.matmul(pxs[tb][:], wsb1[:, kc, :],
                                             xT_sb[:, kc, sl],
                                             start=(kc == 0), stop=(kc == 7))
                            nc.tensor.matmul(pys[tb][:], wsb2[:, kc, :],
                                             xT_sb[:, kc, sl],
                                             start=(kc == 0), stop=(kc == 7))
                    for tb in range(NQB):
                        nc.scalar.copy(
                            stage_dst[:, 1 + QB * tb:1 + QB * (tb + 1)],
                            pxs[tb][:])
                        emit_products(tb, stage_dst, pys[tb])

                def _prod1(tb, stg, py):
                    gsl = slice(QB * tb, QB * (tb + 1))
                    # write rows: shifted stage read; read rows: normal
                    nc.vector.tensor_mul(t1[0:32, gsl],
                                         stg[0:32, QB * tb:QB * (tb + 1)],
                                         py[0:32, :])
                    nc.vector.tensor_mul(t1[32:64, gsl],
                                         stg[32:64, 1 + QB * tb:1 + QB * (tb + 1)],
                                         py[32:64, :])

                def _prod2(tb, stg, py):
                    gsl = slice(QB * tb, QB * (tb + 1))
                    p2 = lns.tile([64, QB], F32, tag="p2")
                    nc.vector.tensor_mul(p2[0:32, :],
                                         stg[0:32, QB * tb:QB * (tb + 1)],
                                         py[0:32, :])
                    nc.vector.tensor_mul(p2[32:64, :],
                                         stg[32:64, 1 + QB * tb:1 + QB * (tb + 1)],
                                         py[32:64, :])
                    nc.vector.tensor_sub(t1[:, gsl], t1[:, gsl], p2[:])
                    nc.scalar.square(sq[0:64, gsl], t1[:, gsl])

                _lines_round(wla_sb, wlb_sb, PXas, _prod1)
                _lines_round(wlc_sb, wld_sb, PXcs, _prod2)

                # norm + scale + scatter, per block (PSUM from the same pool)
                for tb in range(NQB):
                    gsl = slice(QB * tb, QB * (tb + 1))
                    ps = psp.tile([64, QB], F32, tag="pro")
                    nc.tensor.matmul(ps[:], ssel_sb[:], sq[:, gsl],
                                     start=True, stop=True)
                    ssq = lns.tile([64, QB], F32, tag="n1")
                    nc.vector.tensor_scalar_max(out=ssq[:], in0=ps[:],
                                                scalar1=1e-24)
                    rt = lns.tile([64, QB], F32, tag="n2")
                    nc.scalar.sqrt(rt[:], ssq[:])
                    inv = lns.tile([64, QB], F32, tag="n1")
                    nc.vector.reciprocal_approx_fast(out=inv[:], in_=rt[:])
                    # fold inc_scale into read-line norms (rows 0:32 are 1.0)
                    nc.vector.tensor_scalar_mul(out=inv[:], in0=inv[:],
                                                scalar1=sbc_sb[:, 0:1])
                    t1b = lns.tile([64, QB], BF16, tag="n2")
                    nc.vector.tensor_mul(t1b[:], t1[:, gsl], inv[:])
                    for h in range(NH):
                        nc.sync.dma_start(out=jwT[32 * h:32 * h + 6, gsl],
                                          in_=t1b[6 * h:6 * h + 6, :])
                        nc.sync.dma_start(out=rlT[32 * h:32 * h + 6, gsl],
                                          in_=t1b[32 + 6 * h:32 + 6 * h + 6, :])

            # ================= Main loop: per q-block tb =================
            with (
                tc.tile_pool(name="psS", bufs=4,
                             space=bass.MemorySpace.PSUM) as psS,
                tc.tile_pool(name="psL", bufs=2,
                             space=bass.MemorySpace.PSUM) as psL,
                tc.tile_pool(name="ptile", bufs=4) as pp,
                tc.tile_pool(name="rows", bufs=3) as rowp,
                tc.tile_pool(name="uwork", bufs=2) as uwp,
                tc.tile_pool(name="outp", bufs=2) as otp,
            ):
                for tb in range(NQB):
                    gsl = slice(QB * tb, QB * (tb + 1))
                    xsl = slice(1 + QB * tb, 1 + QB * (tb + 1))

                    # ---- A2a: q / k for this block ----
                    for mc in range(2):
                        for (wsb, bias, isq) in ((wq_sb, bq_sb, True),
                                                 (wk_sb, bk_sb, False)):
                            ps = psS.tile([128, QB], F32, tag="s")
                            for kc in range(NKC):
                                nc.tensor.matmul(
                                    ps[:], wsb[:, kc, 128 * mc:128 * (mc + 1)],
                                    xT_sb[:, kc, xsl],
                                    start=(kc == 0), stop=(kc == 7))
                            if isq:
                                nc.vector.tensor_scalar_add(
                                    out=qT[:, mc, gsl], in0=ps[:],
                                    scalar1=bias[:, mc:mc + 1])
                            else:
                                nc.vector.tensor_scalar_add(
                                    out=kTp[0:64, 2 * mc, gsl],
                                    in0=ps[0:64, :],
                                    scalar1=bias[0:64, mc:mc + 1])
                                nc.vector.tensor_scalar_add(
                                    out=kTp[64:128, 2 * mc + 1, gsl],
                                    in0=ps[64:128, :],
                                    scalar1=bias[64:128, mc:mc + 1])

                    # ---- A2b: v|gv stacked ----
                    for ti in range(4 * tb, 4 * tb + 4):
                        ps = psS.tile([128, 512], F32, tag="s")
                        nc.tensor.matmul(ps[:], ones_bf[0:1, :], bvg_sb[:],
                                         start=True, stop=False)
                        for kc in range(NKC):
                            nc.tensor.matmul(
                                ps[:],
                                xT_sb[:, kc, 1 + 128 * ti:1 + 128 * (ti + 1)],
                                wvg_sb[:, kc, :],
                                start=False, stop=(kc == 7))
                        nc.vector.tensor_copy(
                            vplus[:, ti, :].rearrange(
                                "p (h c) -> p h c", c=65)[:, :, 1:65],
                            ps[:, 0:256].rearrange("p (h c) -> p h c", c=64))
                        nc.vector.tensor_copy(
                            gvplus[:, ti, :].rearrange(
                                "p (h c) -> p h c", c=65)[:, :, 1:65],
                            ps[:, 256:512].rearrange("p (h c) -> p h c", c=64))

                    # ---- gate ----
                    psg = psS.tile([16, QB], F32, tag="s")
                    for kc in range(NKC):
                        nc.tensor.matmul(psg[:], wgate_sb[:, kc, :],
                                         xT_sb[:, kc, xsl],
                                         start=(kc == 0), stop=(kc == 7))
                    gsig = rowp.tile([16, QB], BF16, tag="gs")
                    nc.scalar.activation(out=gsig[:], in_=psg[:],
                                         func=AF.Sigmoid,
                                         bias=bgate_sb[:, 0:1], scale=1.0)
                    psm = psS.tile([1, QB], F32, tag="s")
                    nc.tensor.matmul(psm[:], gsel[:], gsig[:],
                                     start=True, stop=True)
                    nc.vector.tensor_copy(g_row[:, gsl], psm[:])
                    nc.vector.tensor_scalar(
                        out=g1m_row[:, gsl], in0=psm[:],
                        scalar1=-1.0, scalar2=1.0, op0=OP.mult, op1=OP.add)

                    # ---- B: dual-path attention, q-block j = tb ----
                    j = tb
                    j2 = j % 2
                    nkt = 4 * (j + 1)
                    for p in range(NH):
                        Us = psS.tile([65, QB], F32, tag="s")
                        Ug = psS.tile([65, QB], F32, tag="s")
                        for kt in range(nkt):
                            m = kt - 4 * j
                            c0 = KT * m if m > 0 else 0
                            ksl = slice(KT * kt, KT * (kt + 1))
                            LB = psL.tile([128, 2, QB], F32, tag="L")
                            nc.tensor.matmul(
                                LB[:, 0, c0:QB], kTp[:, p, ksl],
                                qT[:, p // 2, QB * j + c0:QB * (j + 1)],
                                start=True, stop=True)
                            nc.tensor.matmul(
                                LB[:, 1, c0:QB], jwT[32 * p:32 * p + 6, ksl],
                                rlT[32 * p:32 * p + 6, QB * j + c0:QB * (j + 1)],
                                start=True, stop=True,
                                tile_position=(32 * p, 0))
                            P = pp.tile([128, 2, QB], BF16, tag="P")
                            nc.scalar.activation(P[:, :, c0:QB],
                                                 LB[:, :, c0:QB], AF.Exp)
                            if m >= 0:
                                for path in range(2):
                                    # keep where (col-c0) - chan >= 0
                                    nc.gpsimd.affine_select(
                                        out=P[:, path, c0:c0 + KT],
                                        in_=P[:, path, c0:c0 + KT],
                                        compare_op=OP.is_ge, fill=0.0,
                                        base=0, pattern=[[1, KT]],
                                        channel_multiplier=-1)
                            nc.tensor.matmul(
                                Us[:, c0:QB],
                                vplus[:, kt, 65 * p:65 * p + 65],
                                P[:, 0, c0:QB],
                                start=(kt == 0), stop=(kt == nkt - 1))
                            nc.tensor.matmul(
                                Ug[:, c0:QB],
                                gvplus[:, kt, 65 * p:65 * p + 65],
                                P[:, 1, c0:QB],
                                start=(kt == 0), stop=(kt == nkt - 1))
                        # combine: alpha=(1-g)/Dstd, beta=g/Dgeo; denominators
                        # sit at PSUM partition 0 (ones col 0 of v groups).
                        d0 = rowp.tile([1, QB], F32, tag="r0")
                        e0 = rowp.tile([1, QB], F32, tag="r0")
                        nc.vector.tensor_copy(d0[:], Us[0:1, :])
                        nc.vector.tensor_copy(e0[:], Ug[0:1, :])
                        rs = rowp.tile([1, QB], F32, tag="r1")
                        rg = rowp.tile([1, QB], F32, tag="r1")
                        nc.vector.reciprocal_approx_fast(out=rs[:], in_=d0[:])
                        nc.vector.reciprocal_approx_fast(out=rg[:], in_=e0[:])
                        ar = rowp.tile([1, QB], BF16, tag="r2")
                        br = rowp.tile([1, QB], BF16, tag="r2")
                        nc.vector.tensor_mul(ar[:], rs[:], g1m_row[:, gsl])
                        nc.vector.tensor_mul(br[:], rg[:], g_row[:, gsl])
                        psA = psS.tile([65, QB], F32, tag="s")
                        psB = psS.tile([65, QB], F32, tag="s")
                        nc.tensor.matmul(psA[:], ones_bf[0:1, 0:65], ar[:],
                                         start=True, stop=True)
                        nc.tensor.matmul(psB[:], ones_bf[0:1, 0:65], br[:],
                                         start=True, stop=True)
                        aB = uwp.tile([65, QB], F32, tag="ab")
                        bB = uwp.tile([65, QB], F32, tag="ab")
                        nc.scalar.copy(aB[:], psA[:])
                        nc.scalar.copy(bB[:], psB[:])
                        u1 = uwp.tile([65, QB], F32, tag="u")
                        u2 = uwp.tile([65, QB], F32, tag="u")
                        # partition starts must be quadrant-aligned: include
                        # row 0 (denom*alpha junk — finite, killed by the
                        # zero row 0 of out_w in phase C)
                        nc.vector.tensor_mul(u1[:], Us[:], aB[:])
                        nc.vector.tensor_mul(u2[:], Ug[:], bB[:])
                        nc.vector.tensor_add(comb[0:65, j2, p, :],
                                             u1[:], u2[:])

                    # ---- C: final projection for q-block j ----
                    for qt in range(4):
                        for et in range(2):
                            ps = psS.tile([128, QB], F32, tag="s")
                            for hc in range(NH):
                                nc.tensor.matmul(
                                    ps[:],
                                    comb[:, j2, hc, 128 * qt:128 * (qt + 1)],
                                    outw_sb[:, hc, QB * et:QB * (et + 1)],
                                    start=(hc == 0), stop=(hc == 3))
                            ot = otp.tile([128, QB], F32, tag="o")
                            nc.scalar.copy(ot[:], ps[:])
                            nc.sync.dma_start(
                                d_partial[QB * j + 128 * qt:
                                          QB * j + 128 * (qt + 1),
                                          QB * et:QB * (et + 1)],
                                ot[:])
    nc.compile()
    return nc


_nc_cache = None


def _get_nc():
    global _nc_cache
    if _nc_cache is None:
        _nc_cache = _build_nc()
    return _nc_cache


def _kc_layout(w):
    """[D, C] -> [128, NKC*C] with row (p, kc) = D-index kc*128+p."""
    Dd, C = w.shape
    return np.ascontiguousarray(
        w.reshape(NKC, 128, C).transpose(1, 0, 2).reshape(128, NKC * C))


def _prep_core_inputs(inputs, core):
    b = core // 4
    h0 = (core % 4) * 4
    f = np.float32
    bf = ml_dtypes.bfloat16
    qkv_w, qkv_b = inputs['qkv_w'], inputs['qkv_b']
    scale = DH ** -0.5
    s = slice(h0 * DH, h0 * DH + NH * DH)
    ac = np.ascontiguousarray

    # Operand layout: 64 cols = [write(24)+pad8 | read(24)+pad8]; A/C from w1
    # (shifted = x_prev side), B/D from the w2/read counterparts. The J6
    # contraction is folded into the write gather (reversed pairs + signs).
    WLA = np.zeros((D, 64), f); WLB = np.zeros((D, 64), f)
    WLC = np.zeros((D, 64), f); WLD = np.zeros((D, 64), f)
    w1w, w2w = inputs['w1_write'], inputs['w2_write']
    w1r, w2r = inputs['w1_read'], inputs['w2_read']
    for h in range(NH):
        gh = h0 + h
        for jj in range(6):
            i_, j_ = PAIRS4[5 - jj]
            WLA[:, 0 + h * 6 + jj] = w1w[:, gh * 4 + i_] * SIGMA[jj]    # A_w
            WLB[:, 0 + h * 6 + jj] = w2w[:, gh * 4 + j_]                # B_w
            WLC[:, 0 + h * 6 + jj] = w1w[:, gh * 4 + j_] * SIGMA[jj]    # C_w
            WLD[:, 0 + h * 6 + jj] = w2w[:, gh * 4 + i_]                # D_w
        for pp_ in range(6):
            i_, j_ = PAIRS4[pp_]
            WLA[:, 32 + h * 6 + pp_] = w1r[:, gh * 4 + i_]              # A_r
            WLB[:, 32 + h * 6 + pp_] = w2r[:, gh * 4 + j_]              # B_r
            WLC[:, 32 + h * 6 + pp_] = w1r[:, gh * 4 + j_]              # C_r
            WLD[:, 32 + h * 6 + pp_] = w2r[:, gh * 4 + i_]              # D_r

    ssel = np.zeros((128, 64), f)
    for half in (0, 32):
        for h in range(NH):
            g = slice(half + 6 * h, half + 6 * h + 6)
            ssel[g, g] = 1.0
    sbc = np.ones((64, 1), f)
    sbc[32:56, 0] = np.repeat(inputs['inc_scale'][h0:h0 + NH], 6).astype(f)

    # x^T with a leading zero column per kc chunk (x_prev shift support)
    xT = np.asarray(inputs['x'][b], f).T            # [D, T]
    xTp = np.zeros((NKC, 128, T + 1), f)
    xTp[:, :, 1:] = xT.reshape(NKC, 128, T)
    xTp = xTp.transpose(1, 0, 2).reshape(128, NKC * (T + 1))

    # out_w padded to the comb lhsT layout: rows 1:65 real, 0/65:128 zero
    outw = np.zeros((128, NH, D), f)
    for hc in range(NH):
        outw[1:65, hc, :] = inputs['out_w'][(h0 + hc) * DH:(h0 + hc + 1) * DH, :]

    wv = qkv_w[:, 2 * D:3 * D][:, s].astype(f)
    wgv = inputs['geo_w'][:, s].astype(f)

    return {
        'xT': ac(xTp).astype(bf),
        'wla': _kc_layout(WLA).astype(bf),
        'wlb': _kc_layout(WLB).astype(bf),
        'wlc': _kc_layout(WLC).astype(bf),
        'wld': _kc_layout(WLD).astype(bf),
        'wq': _kc_layout((qkv_w[:, 0 * D:1 * D][:, s] * scale).astype(f)).astype(bf),
        'wk': _kc_layout(qkv_w[:, 1 * D:2 * D][:, s].astype(f)).astype(bf),
        'wvg': _kc_layout(np.concatenate([wv, wgv], axis=1)).astype(bf),
        'wgate': _kc_layout(inputs['gate_w'].astype(f)).astype(bf),
        'outw': ac(outw.reshape(128, NH * D)).astype(bf),
        'ssel': ssel.astype(bf),
        'bq': ac((qkv_b[0 * D:1 * D][s] * scale).astype(f)
                 .reshape(2, 128).transpose(1, 0)),
        'bk': ac(qkv_b[1 * D:2 * D][s].astype(f).reshape(2, 128).transpose(1, 0)),
        'bvg': ac(np.concatenate(
            [qkv_b[2 * D:3 * D][s], inputs['geo_b'][s]]).astype(f)
            .reshape(1, 512)).astype(bf),
        'bgate': ac(inputs['gate_b'].astype(f).reshape(16, 1)),
        'sbc': sbc,
    }


def kernel(**inputs):
    global LAST_RESULT
    inputs = {k: np.asarray(v) for k, v in inputs.items()}
    nc = _get_nc()
    in_maps = [_prep_core_inputs(inputs, c) for c in range(NCORES)]
    res = run_bass_kernel_spmd(nc, in_maps, core_ids=list(range(NCORES)),
                               trace=TRACE)
    LAST_RESULT = res
    out = np.zeros((B, T, D), np.float32)
    for c in range(NCORES):
        out[c // 4] += res.results[c]['partial']
    out += np.asarray(inputs['out_b'], np.float32)[None, None, :]
    return out
